# revision 6
# baseline (speedup 1.0000x reference)
"""CheckersGPT dense transformer forward pass on 8 Trainium2 NeuronCores.

Strategy: pure data-parallel over the batch dim (16 batches -> 2 per core).
Each core runs the full 6-layer transformer on its 512 tokens (2 batches x
256 tokens) with all weights replicated. No collectives needed; the final
[2, 512] probability slices are concatenated on the host.

Layout convention per core (P=128 partitions):
  xT   [128, 4, 512]  : x transposed; chunk c holds embed dims [128c,128c+128),
                        free dim = 512 tokens. Used as matmul rhs/lhsT.
  xN   [128, 4, 512]  : x natural; chunk c holds tokens [128c,128c+128),
                        free dim = 512 embed. Used for residuals / LN.
All matmuls are out = lhsT.T @ rhs with contraction on the partition dim.
"""

import os
import numpy as np
from contextlib import ExitStack

import concourse.bass as bass
import concourse.tile as tile
from concourse import bacc, mybir
from concourse.bass_utils import run_bass_kernel_spmd

F32 = mybir.dt.float32
I32 = mybir.dt.int32
AX = mybir.AxisListType
ALU = mybir.AluOpType
ACTF = mybir.ActivationFunctionType

V, E, L, H, B, T = 512, 512, 6, 8, 16, 256
NCORES = 8
BPC = B // NCORES          # batches per core
TOK = BPC * T              # tokens per core
P = 128
EC = E // P                # embed chunks of 128
TC = TOK // P              # token chunks of 128
NEG = -1e9
EPS = 1e-5

# matmul input dtype: float32r runs the PE at 1 cycle/row (vs 4 for float32)
# for moving dims >= 256, at slightly reduced multiply precision.
MM_DT = mybir.dt.float32r if os.environ.get("CKGPT_MM_DT", "f32r") == "f32r" else F32

_CACHE = {}


def _mm(nc, out, lhsT, rhs, start, stop):
    nc.tensor.matmul(
        out, lhsT.bitcast(MM_DT), rhs.bitcast(MM_DT), start=start, stop=stop
    )


def _build(nlayers=L):
    nc = bacc.Bacc("TRN2", target_bir_lowering=False, debug=False, num_devices=NCORES)

    def din(name, shape, dtype=F32):
        return nc.dram_tensor(name, list(shape), dtype, kind="ExternalInput").ap()

    tok = din("tok", [P, TC], I32)            # token ids, p-major within chunks
    emb = din("emb", [V, E])
    pe2 = din("pe2", [TOK, E])                # positional encoding tiled over BPC
    wq = din("wq", [L, H, E, E])
    wk = din("wk", [L, H, E, E])
    wv = din("wv", [L, H, E, E])
    wo = din("wo", [L, H * E, E])
    bo = din("bo", [L, E])
    ln1w = din("ln1w", [L, E])
    ln1b = din("ln1b", [L, E])
    ln2w = din("ln2w", [L, E])
    ln2b = din("ln2b", [L, E])
    ff1w = din("ff1w", [L, E, E])
    ff1b = din("ff1b", [L, E])
    ff2w = din("ff2w", [L, E, E])
    ff2b = din("ff2b", [L, E])
    wout = din("wout", [E, V])
    bout = din("bout", [V])
    masks = din("masks", [2, P, T])           # additive causal mask per i-chunk
    ident = din("ident", [P, P])
    probs = nc.dram_tensor("probs", [BPC, V], F32, kind="ExternalOutput").ap()

    with tile.TileContext(nc) as tc, ExitStack() as ctx:
        ep = ctx.enter_context

        const = ep(tc.tile_pool(name="const", bufs=1))
        wqkv_p = ep(tc.tile_pool(name="wqkv", bufs=3))
        wo_p = ep(tc.tile_pool(name="wo", bufs=2))
        wff_p = ep(tc.tile_pool(name="wff", bufs=1))
        bias_p = ep(tc.tile_pool(name="bias", bufs=1))
        act_p = ep(tc.tile_pool(name="act", bufs=2))
        qkv_p = ep(tc.tile_pool(name="qkvact", bufs=4))
        ot_p = ep(tc.tile_pool(name="ot", bufs=2))
        ff_p = ep(tc.tile_pool(name="ffact", bufs=3))
        tmp_p = ep(tc.tile_pool(name="tmp", bufs=2))
        esb_p = ep(tc.tile_pool(name="esb", bufs=4))
        attT_p = ep(tc.tile_pool(name="attT", bufs=2))
        st_p = ep(tc.tile_pool(name="stats", bufs=8))
        out_p = ep(tc.tile_pool(name="outp", bufs=1))

        ppb = ep(tc.tile_pool(name="ppb", bufs=3, space="PSUM"))
        ppa = ep(tc.tile_pool(name="ppa", bufs=3, space="PSUM"))
        ppt = ep(tc.tile_pool(name="ppt", bufs=2, space="PSUM"))

        # ---- constants ----
        ident_t = const.tile([P, P], F32)
        nc.sync.dma_start(out=ident_t[:], in_=ident)
        mask_t = const.tile([P, 2, T], F32)
        nc.sync.dma_start(out=mask_t[:], in_=masks.rearrange("c p j -> p c j"))
        eps_t = const.tile([P, 1], F32)
        nc.vector.memset(eps_t[:], EPS)
        tok_t = const.tile([P, TC], I32)
        nc.sync.dma_start(out=tok_t[:], in_=tok)

        def wtile(pool, dram2d, tag):
            t = pool.tile([P, EC, E], F32, tag=tag)
            nc.sync.dma_start(
                out=t[:].bitcast(MM_DT),
                in_=dram2d.rearrange("(c p) o -> p c o", p=P).bitcast(MM_DT),
            )
            return t

        def bbcast(vec_ap, tag="bias"):
            t = bias_p.tile([P, E], F32, tag=tag)
            nc.sync.dma_start(out=t[:], in_=vec_ap.partition_broadcast(P))
            return t

        def transpose_into(dstT, srcN, tag, mm_dst=True):
            # dstT[:, bb, a*P:(a+1)*P] = srcN[:, a, bb*P:(bb+1)*P].T
            for a in range(TC):
                for bb in range(EC):
                    tp = ppt.tile([P, P], F32, tag="tp")
                    nc.tensor.transpose(
                        tp[:], srcN[:, a, bb * P : (bb + 1) * P], ident_t[:]
                    )
                    dst = dstT[:, bb, a * P : (a + 1) * P]
                    if mm_dst:
                        dst = dst.bitcast(MM_DT)
                    if (a + bb) % 2 == 0:
                        nc.vector.tensor_copy(dst, tp[:])
                    else:
                        nc.scalar.copy(dst, tp[:])

        def layernorm(src, dst, w_b, b_b, tag):
            # dst = (src - mean)/sqrt(var+eps) * w + b ; src/dst [128, E]
            stt = st_p.tile([P, 6], F32, tag=tag + "s")
            nc.vector.bn_stats(out=stt[:], in_=src)
            mv = st_p.tile([P, 2], F32, tag=tag + "m")
            nc.vector.bn_aggr(out=mv[:], in_=stt[:])
            sd = st_p.tile([P, 1], F32, tag=tag + "d")
            nc.scalar.activation(
                out=sd[:], in_=mv[:, 1:2], func=ACTF.Sqrt, bias=eps_t[:, 0:1]
            )
            rs = st_p.tile([P, 1], F32, tag=tag + "r")
            nc.vector.reciprocal(out=rs[:], in_=sd[:])
            nc.vector.tensor_scalar(
                out=dst,
                in0=src,
                scalar1=mv[:, 0:1],
                scalar2=rs[:],
                op0=ALU.subtract,
                op1=ALU.mult,
            )
            nc.gpsimd.tensor_mul(out=dst, in0=dst, in1=w_b[:])
            nc.gpsimd.tensor_add(out=dst, in0=dst, in1=b_b[:])

        # ---- embedding gather + positional encoding ----
        xN = act_p.tile([P, TC, E], F32, tag="xN")
        for c in range(TC):
            nc.gpsimd.indirect_dma_start(
                out=xN[:, c, :],
                out_offset=None,
                in_=emb,
                in_offset=bass.IndirectOffsetOnAxis(ap=tok_t[:, c : c + 1], axis=0),
            )
        pe_t = ff_p.tile([P, TC, E], F32, tag="ff")
        nc.sync.dma_start(out=pe_t[:], in_=pe2.rearrange("(c p) o -> p c o", p=P))
        for c in range(TC):
            nc.vector.tensor_add(
                out=xN[:, c, :], in0=xN[:, c, :], in1=pe_t[:, c, :]
            )
        xT = act_p.tile([P, EC, TOK], F32, tag="xT")
        transpose_into(xT, xN, "x0T")

        for l in range(nlayers):
            bo_b = bbcast(bo[l], "b_bo")
            ln1w_b = bbcast(ln1w[l], "b_l1w")
            ln1b_b = bbcast(ln1b[l], "b_l1b")
            ln2w_b = bbcast(ln2w[l], "b_l2w")
            ln2b_b = bbcast(ln2b[l], "b_l2b")
            ff2b_b = bbcast(ff2b[l], "b_f2")
            ff1b_t = bias_p.tile([P, EC], F32, tag="b_f1")
            nc.sync.dma_start(
                out=ff1b_t[:], in_=ff1b[l].rearrange("(c p) -> p c", p=P)
            )

            attn_acc = act_p.tile([P, TC, E], F32, tag="acc")

            for h in range(H):
                wq_t = wtile(wqkv_p, wq[l, h], "wqkv")
                wk_t = wtile(wqkv_p, wk[l, h], "wqkv")
                wv_t = wtile(wqkv_p, wv[l, h], "wqkv")
                wo_t = wtile(wo_p, wo[l, h * E : (h + 1) * E, :], "wo")

                # Q^T, K^T: [o-chunk, tokens] ; V natural: [t-chunk, o]
                QT = qkv_p.tile([P, EC, TOK], F32, tag="qkv")
                KT = qkv_p.tile([P, EC, TOK], F32, tag="qkv")
                VN = qkv_p.tile([P, TC, E], F32, tag="qkv")
                for oc in range(EC):
                    ps = ppb.tile([P, TOK], F32, tag="ppb")
                    for ec in range(EC):
                        _mm(nc, ps[:], wq_t[:, ec, oc * P : (oc + 1) * P],
                            xT[:, ec, :], ec == 0, ec == EC - 1)
                    nc.scalar.copy(QT[:, oc, :].bitcast(MM_DT), ps[:])
                for oc in range(EC):
                    ps = ppb.tile([P, TOK], F32, tag="ppb")
                    for ec in range(EC):
                        _mm(nc, ps[:], wk_t[:, ec, oc * P : (oc + 1) * P],
                            xT[:, ec, :], ec == 0, ec == EC - 1)
                    nc.scalar.copy(KT[:, oc, :].bitcast(MM_DT), ps[:])
                for tcc in range(TC):
                    ps = ppb.tile([P, E], F32, tag="ppb")
                    for ec in range(EC):
                        _mm(nc, ps[:], xT[:, ec, tcc * P : (tcc + 1) * P],
                            wv_t[:, ec, :], ec == 0, ec == EC - 1)
                    nc.vector.tensor_copy(VN[:, tcc, :].bitcast(MM_DT), ps[:])

                for b in range(BPC):
                    t0 = b * T
                    att_sb = []
                    for icl in range(2):
                        ic = 2 * b + icl
                        pse = ppa.tile([P, T], F32, tag="ppa")
                        for ec in range(EC):
                            _mm(nc, pse[:], QT[:, ec, ic * P : (ic + 1) * P],
                                KT[:, ec, t0 : t0 + T], ec == 0, ec == EC - 1)
                        esb = esb_p.tile([P, T], F32, tag="esb")
                        nc.vector.tensor_add(out=esb[:], in0=pse[:], in1=mask_t[:, icl, :])
                        nmax = st_p.tile([P, 1], F32, tag="nmax")
                        nc.vector.reduce_max(
                            out=nmax[:], in_=esb[:], axis=AX.X, negate=True
                        )
                        den = st_p.tile([P, 1], F32, tag="den")
                        nc.scalar.activation(
                            out=esb[:], in_=esb[:], func=ACTF.Exp,
                            bias=nmax[:, 0:1], accum_out=den[:],
                        )
                        rec = st_p.tile([P, 1], F32, tag="rec")
                        nc.vector.reciprocal(out=rec[:], in_=den[:])
                        nc.gpsimd.tensor_scalar_mul(
                            out=esb[:], in0=esb[:], scalar1=rec[:]
                        )
                        att_sb.append(esb)

                    attT = attT_p.tile([P, 2, T], F32, tag="attT")
                    for icl in range(2):
                        for jcl in range(2):
                            tp = ppt.tile([P, P], F32, tag="tp")
                            nc.tensor.transpose(
                                tp[:], att_sb[icl][:, jcl * P : (jcl + 1) * P],
                                ident_t[:],
                            )
                            adst = attT[:, jcl, icl * P : (icl + 1) * P].bitcast(MM_DT)
                            if (icl + jcl) % 2 == 0:
                                nc.vector.tensor_copy(adst, tp[:])
                            else:
                                nc.scalar.copy(adst, tp[:])

                    oTb = ot_p.tile([P, EC, T], F32, tag="oT")
                    for ec in range(EC):
                        po = ppa.tile([P, T], F32, tag="ppa")
                        for jcl in range(2):
                            _mm(nc, po[:], VN[:, 2 * b + jcl, ec * P : (ec + 1) * P],
                                attT[:, jcl, :], jcl == 0, jcl == 1)
                        nc.vector.tensor_copy(oTb[:, ec, :].bitcast(MM_DT), po[:])

                    for tcl in range(2):
                        tcc = 2 * b + tcl
                        pw = ppb.tile([P, E], F32, tag="ppb")
                        for ec in range(EC):
                            _mm(nc, pw[:], oTb[:, ec, tcl * P : (tcl + 1) * P],
                                wo_t[:, ec, :], ec == 0, ec == EC - 1)
                        if h == 0:
                            nc.vector.tensor_copy(attn_acc[:, tcc, :], pw[:])
                        else:
                            nc.vector.tensor_add(
                                out=attn_acc[:, tcc, :],
                                in0=attn_acc[:, tcc, :], in1=pw[:],
                            )

            # ---- FFN block ----
            ff1w_t = wtile(wff_p, ff1w[l], "wff1")
            ff2w_t = wtile(wff_p, ff2w[l], "wff2")

            h1N = ff_p.tile([P, TC, E], F32, tag="ff")
            for tcc in range(TC):
                # attn_acc becomes attn_out (+bo); keep for second residual
                nc.gpsimd.tensor_add(
                    out=attn_acc[:, tcc, :], in0=attn_acc[:, tcc, :], in1=bo_b[:]
                )
                s1 = tmp_p.tile([P, E], F32, tag="s1")
                nc.gpsimd.tensor_add(
                    out=s1[:], in0=attn_acc[:, tcc, :], in1=xN[:, tcc, :]
                )
                layernorm(s1[:], h1N[:, tcc, :], ln1w_b, ln1b_b, "ln1")

            h1T = ff_p.tile([P, EC, TOK], F32, tag="ff")
            transpose_into(h1T, h1N, "h1T")

            r1T = ff_p.tile([P, EC, TOK], F32, tag="ff")
            for fc in range(EC):
                ps = ppb.tile([P, TOK], F32, tag="ppb")
                for ec in range(EC):
                    _mm(nc, ps[:], ff1w_t[:, ec, fc * P : (fc + 1) * P],
                        h1T[:, ec, :], ec == 0, ec == EC - 1)
                nc.scalar.activation(
                    out=r1T[:, fc, :].bitcast(MM_DT), in_=ps[:], func=ACTF.Relu,
                    bias=ff1b_t[:, fc : fc + 1],
                )

            xN_new = act_p.tile([P, TC, E], F32, tag="xN")
            for tcc in range(TC):
                ps = ppb.tile([P, E], F32, tag="ppb")
                for fc in range(EC):
                    _mm(nc, ps[:], r1T[:, fc, tcc * P : (tcc + 1) * P],
                        ff2w_t[:, fc, :], fc == 0, fc == EC - 1)
                s2 = tmp_p.tile([P, E], F32, tag="s1")
                nc.vector.tensor_add(out=s2[:], in0=ps[:], in1=ff2b_b[:])
                nc.gpsimd.tensor_add(
                    out=s2[:], in0=s2[:], in1=attn_acc[:, tcc, :]
                )
                layernorm(s2[:], xN_new[:, tcc, :], ln2w_b, ln2b_b, "ln2")
            xN = xN_new
            xT = act_p.tile([P, EC, TOK], F32, tag="xT")
            transpose_into(xT, xN, "xT")

        # ---- output head: last token of each batch ----
        wout_t = wtile(wo_p, wout, "wo")
        bout_t = out_p.tile([BPC, V], F32)
        nc.sync.dma_start(out=bout_t[:], in_=bout.partition_broadcast(BPC))
        pl = ppb.tile([BPC, V], F32, tag="ppb")
        for ec in range(EC):
            _mm(nc, pl[:], xT[:, ec, T - 1 :: T], wout_t[:, ec, :],
                ec == 0, ec == EC - 1)
        logits = out_p.tile([BPC, V], F32)
        nc.vector.tensor_add(out=logits[:], in0=pl[:], in1=bout_t[:])
        nmax = out_p.tile([BPC, 1], F32)
        nc.vector.reduce_max(out=nmax[:], in_=logits[:], axis=AX.X, negate=True)
        den = out_p.tile([BPC, 1], F32)
        nc.scalar.activation(
            out=logits[:], in_=logits[:], func=ACTF.Exp,
            bias=nmax[:, 0:1], accum_out=den[:],
        )
        rec = out_p.tile([BPC, 1], F32)
        nc.vector.reciprocal(out=rec[:], in_=den[:])
        nc.vector.tensor_scalar_mul(out=logits[:], in0=logits[:], scalar1=rec[:])
        nc.sync.dma_start(out=probs, in_=logits[:])

    nc.compile()
    return nc


def _pe_table():
    i = np.arange(E, dtype=np.float32)
    rates = (1.0 / np.power(np.float32(10000.0), 2.0 * np.floor(i / 2.0) / E)).astype(
        np.float32
    )
    ang = np.arange(T, dtype=np.float32)[:, None] * rates[None, :]
    pe = np.concatenate([np.sin(ang[:, 0::2]), np.cos(ang[:, 1::2])], axis=-1)
    return np.tile(pe.astype(np.float32), (BPC, 1))  # [TOK, E]


def _masks():
    m = np.zeros((2, P, T), dtype=np.float32)
    j = np.arange(T)
    for c in range(2):
        i = c * P + np.arange(P)
        m[c] = np.where(j[None, :] > i[:, None], np.float32(NEG), np.float32(0.0))
    return m


def _prep_in_maps(
    input_tokens, emb, wq, wk, wv, wo, bo, ln1_w, ln1_b, ln2_w, ln2_b,
    ff1_w, ff1_b, ff2_w, ff2_b, wout, bout,
):
    f = lambda x: np.ascontiguousarray(np.asarray(x, dtype=np.float32))
    toks = np.asarray(input_tokens).astype(np.int64)
    shared = {
        "emb": f(emb), "wq": f(wq), "wk": f(wk), "wv": f(wv), "wo": f(wo),
        "bo": f(bo), "ln1w": f(ln1_w), "ln1b": f(ln1_b), "ln2w": f(ln2_w),
        "ln2b": f(ln2_b), "ff1w": f(ff1_w), "ff1b": f(ff1_b), "ff2w": f(ff2_w),
        "ff2b": f(ff2_b), "wout": f(wout), "bout": f(bout),
        "pe2": _pe_table(), "masks": _masks(),
        "ident": np.eye(P, dtype=np.float32),
    }
    in_maps = []
    for c in range(NCORES):
        t = toks[c * BPC : (c + 1) * BPC].reshape(TOK)  # [512] flat tokens
        tokarr = np.ascontiguousarray(t.reshape(TC, P).T.astype(np.int32))
        in_maps.append({**shared, "tok": tokarr})
    return in_maps


def kernel(**inputs):
    if "nc" not in _CACHE:
        _CACHE["nc"] = _build()
    nc = _CACHE["nc"]
    in_maps = _prep_in_maps(**inputs)
    res = run_bass_kernel_spmd(nc, in_maps, core_ids=list(range(NCORES)))
    _CACHE["last_results"] = res
    out = np.concatenate([res.results[c]["probs"] for c in range(NCORES)], axis=0)
    return out.astype(np.float32)


# revision 9
# speedup vs baseline: 34.8449x; 34.8449x over previous
"""CheckersGPT dense transformer forward pass on 8 Trainium2 NeuronCores.

Strategy: pure data-parallel over the batch dim (16 batches -> 2 per core).
Each core runs the full 6-layer transformer on its 512 tokens (2 batches x
256 tokens) with all weights replicated. No collectives needed; the final
[2, 512] probability slices are concatenated on the host.

Layout convention per core (P=128 partitions):
  xT   [128, 4, 512]  : x transposed; chunk c holds embed dims [128c,128c+128),
                        free dim = 512 tokens. Used as matmul rhs/lhsT.
  xN   [128, 4, 512]  : x natural; chunk c holds tokens [128c,128c+128),
                        free dim = 512 embed. Used for residuals / LN.
All matmuls are out = lhsT.T @ rhs with contraction on the partition dim.
"""

import os
import numpy as np
from contextlib import ExitStack

import concourse.bass as bass
import concourse.tile as tile
from concourse import bacc, mybir
from concourse.bass_utils import run_bass_kernel_spmd

F32 = mybir.dt.float32
I32 = mybir.dt.int32
AX = mybir.AxisListType
ALU = mybir.AluOpType
ACTF = mybir.ActivationFunctionType

V, E, L, H, B, T = 512, 512, 6, 8, 16, 256
NCORES = 8
BPC = B // NCORES          # batches per core
TOK = BPC * T              # tokens per core
P = 128
EC = E // P                # embed chunks of 128
TC = TOK // P              # token chunks of 128
NEG = -1e9
EPS = 1e-5

# matmul input dtype: float32r runs the PE at 1 cycle/row (vs 4 for float32)
# for moving dims >= 256, at slightly reduced multiply precision.
MM_DT = mybir.dt.float32r if os.environ.get("CKGPT_MM_DT", "f32r") == "f32r" else F32

_CACHE = {}


def _mm(nc, out, lhsT, rhs, start, stop):
    nc.tensor.matmul(
        out, lhsT.bitcast(MM_DT), rhs.bitcast(MM_DT), start=start, stop=stop
    )


def _build(nlayers=L, reps=1):
    nc = bacc.Bacc("TRN2", target_bir_lowering=False, debug=False, num_devices=NCORES)

    def din(name, shape, dtype=F32):
        return nc.dram_tensor(name, list(shape), dtype, kind="ExternalInput").ap()

    tok = din("tok", [P, TC], I32)            # token ids, p-major within chunks
    emb = din("emb", [V, E])
    pe2 = din("pe2", [TOK, E])                # positional encoding tiled over BPC
    wq = din("wq", [L, H, E, E])
    wk = din("wk", [L, H, E, E])
    wv = din("wv", [L, H, E, E])
    wo = din("wo", [L, H * E, E])
    bo = din("bo", [L, E])
    ln1w = din("ln1w", [L, E])
    ln1b = din("ln1b", [L, E])
    ln2w = din("ln2w", [L, E])
    ln2b = din("ln2b", [L, E])
    ff1w = din("ff1w", [L, E, E])
    ff1b = din("ff1b", [L, E])
    ff2w = din("ff2w", [L, E, E])
    ff2b = din("ff2b", [L, E])
    wout = din("wout", [E, V])
    bout = din("bout", [V])
    masks = din("masks", [2, P, T])           # additive causal mask per i-chunk
    ident = din("ident", [P, P])
    probs = nc.dram_tensor("probs", [BPC, V], F32, kind="ExternalOutput").ap()

    with tile.TileContext(nc) as tc, ExitStack() as ctx:
        ep = ctx.enter_context

        const = ep(tc.tile_pool(name="const", bufs=1))
        wqkv_p = ep(tc.tile_pool(name="wqkv", bufs=3))
        wo_p = ep(tc.tile_pool(name="wo", bufs=2))
        wff_p = ep(tc.tile_pool(name="wff", bufs=1))
        bias_p = ep(tc.tile_pool(name="bias", bufs=1))
        act_p = ep(tc.tile_pool(name="act", bufs=2))
        qkv_p = ep(tc.tile_pool(name="qkvact", bufs=4))
        ot_p = ep(tc.tile_pool(name="ot", bufs=2))
        ff_p = ep(tc.tile_pool(name="ffact", bufs=3))
        tmp_p = ep(tc.tile_pool(name="tmp", bufs=2))
        esb_p = ep(tc.tile_pool(name="esb", bufs=4))
        attT_p = ep(tc.tile_pool(name="attT", bufs=2))
        st_p = ep(tc.tile_pool(name="stats", bufs=8))
        out_p = ep(tc.tile_pool(name="outp", bufs=1))

        ppb = ep(tc.tile_pool(name="ppb", bufs=3, space="PSUM"))
        ppa = ep(tc.tile_pool(name="ppa", bufs=3, space="PSUM"))
        ppt = ep(tc.tile_pool(name="ppt", bufs=2, space="PSUM"))

        # ---- constants ----
        ident_t = const.tile([P, P], F32)
        nc.sync.dma_start(out=ident_t[:], in_=ident)
        mask_t = const.tile([P, 2, T], F32)
        nc.sync.dma_start(out=mask_t[:], in_=masks.rearrange("c p j -> p c j"))
        eps_t = const.tile([P, 1], F32)
        nc.vector.memset(eps_t[:], EPS)
        tok_t = const.tile([P, TC], I32)
        nc.sync.dma_start(out=tok_t[:], in_=tok)

        def wtile(pool, dram2d, tag):
            t = pool.tile([P, EC, E], F32, tag=tag)
            nc.sync.dma_start(
                out=t[:].bitcast(MM_DT),
                in_=dram2d.rearrange("(c p) o -> p c o", p=P).bitcast(MM_DT),
            )
            return t

        def bbcast(vec_ap, tag="bias"):
            t = bias_p.tile([P, E], F32, tag=tag)
            nc.sync.dma_start(out=t[:], in_=vec_ap.partition_broadcast(P))
            return t

        def transpose_into(dstT, srcN, tag, mm_dst=True):
            # dstT[:, bb, a*P:(a+1)*P] = srcN[:, a, bb*P:(bb+1)*P].T
            for a in range(TC):
                for bb in range(EC):
                    tp = ppt.tile([P, P], F32, tag="tp")
                    nc.tensor.transpose(
                        tp[:], srcN[:, a, bb * P : (bb + 1) * P], ident_t[:]
                    )
                    dst = dstT[:, bb, a * P : (a + 1) * P]
                    if mm_dst:
                        dst = dst.bitcast(MM_DT)
                    if (a + bb) % 2 == 0:
                        nc.vector.tensor_copy(dst, tp[:])
                    else:
                        nc.scalar.copy(dst, tp[:])

        def layernorm(src, dst, w_b, b_b, tag):
            # dst = (src - mean)/sqrt(var+eps) * w + b ; src/dst [128, E]
            stt = st_p.tile([P, 6], F32, tag=tag + "s")
            nc.vector.bn_stats(out=stt[:], in_=src)
            mv = st_p.tile([P, 2], F32, tag=tag + "m")
            nc.vector.bn_aggr(out=mv[:], in_=stt[:])
            sd = st_p.tile([P, 1], F32, tag=tag + "d")
            nc.scalar.activation(
                out=sd[:], in_=mv[:, 1:2], func=ACTF.Sqrt, bias=eps_t[:, 0:1]
            )
            rs = st_p.tile([P, 1], F32, tag=tag + "r")
            nc.vector.reciprocal(out=rs[:], in_=sd[:])
            nc.vector.tensor_scalar(
                out=dst,
                in0=src,
                scalar1=mv[:, 0:1],
                scalar2=rs[:],
                op0=ALU.subtract,
                op1=ALU.mult,
            )
            nc.gpsimd.tensor_mul(out=dst, in0=dst, in1=w_b[:])
            nc.gpsimd.tensor_add(out=dst, in0=dst, in1=b_b[:])

        # ---- whole forward pass (optionally repeated for timing) ----
        def emit_forward():
            emit_body(
                nc, tc, nlayers,
                const, wqkv_p, wo_p, wff_p, bias_p, act_p, qkv_p, ot_p, ff_p,
                tmp_p, esb_p, attT_p, st_p, out_p, ppb, ppa, ppt,
                ident_t, mask_t, eps_t, tok_t,
                wtile, bbcast, transpose_into, layernorm,
                emb, pe2, wq, wk, wv, wo, bo, ln1w, ln1b, ln2w, ln2b,
                ff1w, ff1b, ff2w, ff2b, wout, bout, probs,
            )

        if reps > 1:
            with tc.For_i(0, reps, 1):
                emit_forward()
        else:
            emit_forward()

    nc.compile()
    return nc


def emit_body(
    nc, tc, nlayers,
    const, wqkv_p, wo_p, wff_p, bias_p, act_p, qkv_p, ot_p, ff_p,
    tmp_p, esb_p, attT_p, st_p, out_p, ppb, ppa, ppt,
    ident_t, mask_t, eps_t, tok_t,
    wtile, bbcast, transpose_into, layernorm,
    emb, pe2, wq, wk, wv, wo, bo, ln1w, ln1b, ln2w, ln2b,
    ff1w, ff1b, ff2w, ff2b, wout, bout, probs,
):
    if True:
        # ---- embedding gather + positional encoding ----
        xN = act_p.tile([P, TC, E], F32, tag="xN")
        for c in range(TC):
            nc.gpsimd.indirect_dma_start(
                out=xN[:, c, :],
                out_offset=None,
                in_=emb,
                in_offset=bass.IndirectOffsetOnAxis(ap=tok_t[:, c : c + 1], axis=0),
            )
        pe_t = ff_p.tile([P, TC, E], F32, tag="ff")
        nc.sync.dma_start(out=pe_t[:], in_=pe2.rearrange("(c p) o -> p c o", p=P))
        for c in range(TC):
            nc.vector.tensor_add(
                out=xN[:, c, :], in0=xN[:, c, :], in1=pe_t[:, c, :]
            )
        xT = act_p.tile([P, EC, TOK], F32, tag="xT")
        transpose_into(xT, xN, "x0T")

        for l in range(nlayers):
            bo_b = bbcast(bo[l], "b_bo")
            ln1w_b = bbcast(ln1w[l], "b_l1w")
            ln1b_b = bbcast(ln1b[l], "b_l1b")
            ln2w_b = bbcast(ln2w[l], "b_l2w")
            ln2b_b = bbcast(ln2b[l], "b_l2b")
            ff2b_b = bbcast(ff2b[l], "b_f2")
            ff1b_t = bias_p.tile([P, EC], F32, tag="b_f1")
            nc.sync.dma_start(
                out=ff1b_t[:], in_=ff1b[l].rearrange("(c p) -> p c", p=P)
            )

            attn_acc = act_p.tile([P, TC, E], F32, tag="acc")

            for h in range(H):
                wq_t = wtile(wqkv_p, wq[l, h], "wqkv")
                wk_t = wtile(wqkv_p, wk[l, h], "wqkv")
                wv_t = wtile(wqkv_p, wv[l, h], "wqkv")
                wo_t = wtile(wo_p, wo[l, h * E : (h + 1) * E, :], "wo")

                # Q^T, K^T: [o-chunk, tokens] ; V natural: [t-chunk, o]
                QT = qkv_p.tile([P, EC, TOK], F32, tag="qkv")
                KT = qkv_p.tile([P, EC, TOK], F32, tag="qkv")
                VN = qkv_p.tile([P, TC, E], F32, tag="qkv")
                for oc in range(EC):
                    ps = ppb.tile([P, TOK], F32, tag="ppb")
                    for ec in range(EC):
                        _mm(nc, ps[:], wq_t[:, ec, oc * P : (oc + 1) * P],
                            xT[:, ec, :], ec == 0, ec == EC - 1)
                    nc.scalar.copy(QT[:, oc, :].bitcast(MM_DT), ps[:])
                for oc in range(EC):
                    ps = ppb.tile([P, TOK], F32, tag="ppb")
                    for ec in range(EC):
                        _mm(nc, ps[:], wk_t[:, ec, oc * P : (oc + 1) * P],
                            xT[:, ec, :], ec == 0, ec == EC - 1)
                    nc.scalar.copy(KT[:, oc, :].bitcast(MM_DT), ps[:])
                for tcc in range(TC):
                    ps = ppb.tile([P, E], F32, tag="ppb")
                    for ec in range(EC):
                        _mm(nc, ps[:], xT[:, ec, tcc * P : (tcc + 1) * P],
                            wv_t[:, ec, :], ec == 0, ec == EC - 1)
                    nc.vector.tensor_copy(VN[:, tcc, :].bitcast(MM_DT), ps[:])

                for b in range(BPC):
                    t0 = b * T
                    att_sb = []
                    for icl in range(2):
                        ic = 2 * b + icl
                        pse = ppa.tile([P, T], F32, tag="ppa")
                        for ec in range(EC):
                            _mm(nc, pse[:], QT[:, ec, ic * P : (ic + 1) * P],
                                KT[:, ec, t0 : t0 + T], ec == 0, ec == EC - 1)
                        esb = esb_p.tile([P, T], F32, tag="esb")
                        nc.vector.tensor_add(out=esb[:], in0=pse[:], in1=mask_t[:, icl, :])
                        nmax = st_p.tile([P, 1], F32, tag="nmax")
                        nc.vector.reduce_max(
                            out=nmax[:], in_=esb[:], axis=AX.X, negate=True
                        )
                        den = st_p.tile([P, 1], F32, tag="den")
                        nc.scalar.activation(
                            out=esb[:], in_=esb[:], func=ACTF.Exp,
                            bias=nmax[:, 0:1], accum_out=den[:],
                        )
                        rec = st_p.tile([P, 1], F32, tag="rec")
                        nc.vector.reciprocal(out=rec[:], in_=den[:])
                        nc.gpsimd.tensor_scalar_mul(
                            out=esb[:], in0=esb[:], scalar1=rec[:]
                        )
                        att_sb.append(esb)

                    attT = attT_p.tile([P, 2, T], F32, tag="attT")
                    for icl in range(2):
                        for jcl in range(2):
                            tp = ppt.tile([P, P], F32, tag="tp")
                            nc.tensor.transpose(
                                tp[:], att_sb[icl][:, jcl * P : (jcl + 1) * P],
                                ident_t[:],
                            )
                            adst = attT[:, jcl, icl * P : (icl + 1) * P].bitcast(MM_DT)
                            if (icl + jcl) % 2 == 0:
                                nc.vector.tensor_copy(adst, tp[:])
                            else:
                                nc.scalar.copy(adst, tp[:])

                    oTb = ot_p.tile([P, EC, T], F32, tag="oT")
                    for ec in range(EC):
                        po = ppa.tile([P, T], F32, tag="ppa")
                        for jcl in range(2):
                            _mm(nc, po[:], VN[:, 2 * b + jcl, ec * P : (ec + 1) * P],
                                attT[:, jcl, :], jcl == 0, jcl == 1)
                        nc.vector.tensor_copy(oTb[:, ec, :].bitcast(MM_DT), po[:])

                    for tcl in range(2):
                        tcc = 2 * b + tcl
                        pw = ppb.tile([P, E], F32, tag="ppb")
                        for ec in range(EC):
                            _mm(nc, pw[:], oTb[:, ec, tcl * P : (tcl + 1) * P],
                                wo_t[:, ec, :], ec == 0, ec == EC - 1)
                        if h == 0:
                            nc.vector.tensor_copy(attn_acc[:, tcc, :], pw[:])
                        else:
                            nc.vector.tensor_add(
                                out=attn_acc[:, tcc, :],
                                in0=attn_acc[:, tcc, :], in1=pw[:],
                            )

            # ---- FFN block ----
            ff1w_t = wtile(wff_p, ff1w[l], "wff1")
            ff2w_t = wtile(wff_p, ff2w[l], "wff2")

            h1N = ff_p.tile([P, TC, E], F32, tag="ff")
            for tcc in range(TC):
                # attn_acc becomes attn_out (+bo); keep for second residual
                nc.gpsimd.tensor_add(
                    out=attn_acc[:, tcc, :], in0=attn_acc[:, tcc, :], in1=bo_b[:]
                )
                s1 = tmp_p.tile([P, E], F32, tag="s1")
                nc.gpsimd.tensor_add(
                    out=s1[:], in0=attn_acc[:, tcc, :], in1=xN[:, tcc, :]
                )
                layernorm(s1[:], h1N[:, tcc, :], ln1w_b, ln1b_b, "ln1")

            h1T = ff_p.tile([P, EC, TOK], F32, tag="ff")
            transpose_into(h1T, h1N, "h1T")

            r1T = ff_p.tile([P, EC, TOK], F32, tag="ff")
            for fc in range(EC):
                ps = ppb.tile([P, TOK], F32, tag="ppb")
                for ec in range(EC):
                    _mm(nc, ps[:], ff1w_t[:, ec, fc * P : (fc + 1) * P],
                        h1T[:, ec, :], ec == 0, ec == EC - 1)
                nc.scalar.activation(
                    out=r1T[:, fc, :].bitcast(MM_DT), in_=ps[:], func=ACTF.Relu,
                    bias=ff1b_t[:, fc : fc + 1],
                )

            xN_new = act_p.tile([P, TC, E], F32, tag="xN")
            for tcc in range(TC):
                ps = ppb.tile([P, E], F32, tag="ppb")
                for fc in range(EC):
                    _mm(nc, ps[:], r1T[:, fc, tcc * P : (tcc + 1) * P],
                        ff2w_t[:, fc, :], fc == 0, fc == EC - 1)
                s2 = tmp_p.tile([P, E], F32, tag="s1")
                nc.vector.tensor_add(out=s2[:], in0=ps[:], in1=ff2b_b[:])
                nc.gpsimd.tensor_add(
                    out=s2[:], in0=s2[:], in1=attn_acc[:, tcc, :]
                )
                layernorm(s2[:], xN_new[:, tcc, :], ln2w_b, ln2b_b, "ln2")
            xN = xN_new
            xT = act_p.tile([P, EC, TOK], F32, tag="xT")
            transpose_into(xT, xN, "xT")

        # ---- output head: last token of each batch ----
        wout_t = wtile(wo_p, wout, "wo")
        bout_t = out_p.tile([BPC, V], F32)
        nc.sync.dma_start(out=bout_t[:], in_=bout.partition_broadcast(BPC))
        pl = ppb.tile([BPC, V], F32, tag="ppb")
        for ec in range(EC):
            _mm(nc, pl[:], xT[:, ec, T - 1 :: T], wout_t[:, ec, :],
                ec == 0, ec == EC - 1)
        logits = out_p.tile([BPC, V], F32)
        nc.vector.tensor_add(out=logits[:], in0=pl[:], in1=bout_t[:])
        nmax = out_p.tile([BPC, 1], F32)
        nc.vector.reduce_max(out=nmax[:], in_=logits[:], axis=AX.X, negate=True)
        den = out_p.tile([BPC, 1], F32)
        nc.scalar.activation(
            out=logits[:], in_=logits[:], func=ACTF.Exp,
            bias=nmax[:, 0:1], accum_out=den[:],
        )
        rec = out_p.tile([BPC, 1], F32)
        nc.vector.reciprocal(out=rec[:], in_=den[:])
        nc.vector.tensor_scalar_mul(out=logits[:], in0=logits[:], scalar1=rec[:])
        nc.sync.dma_start(out=probs, in_=logits[:])


def _pe_table():
    i = np.arange(E, dtype=np.float32)
    rates = (1.0 / np.power(np.float32(10000.0), 2.0 * np.floor(i / 2.0) / E)).astype(
        np.float32
    )
    ang = np.arange(T, dtype=np.float32)[:, None] * rates[None, :]
    pe = np.concatenate([np.sin(ang[:, 0::2]), np.cos(ang[:, 1::2])], axis=-1)
    return np.tile(pe.astype(np.float32), (BPC, 1))  # [TOK, E]


def _masks():
    m = np.zeros((2, P, T), dtype=np.float32)
    j = np.arange(T)
    for c in range(2):
        i = c * P + np.arange(P)
        m[c] = np.where(j[None, :] > i[:, None], np.float32(NEG), np.float32(0.0))
    return m


def _prep_in_maps(
    input_tokens, emb, wq, wk, wv, wo, bo, ln1_w, ln1_b, ln2_w, ln2_b,
    ff1_w, ff1_b, ff2_w, ff2_b, wout, bout,
):
    f = lambda x: np.ascontiguousarray(np.asarray(x, dtype=np.float32))
    toks = np.asarray(input_tokens).astype(np.int64)
    shared = {
        "emb": f(emb), "wq": f(wq), "wk": f(wk), "wv": f(wv), "wo": f(wo),
        "bo": f(bo), "ln1w": f(ln1_w), "ln1b": f(ln1_b), "ln2w": f(ln2_w),
        "ln2b": f(ln2_b), "ff1w": f(ff1_w), "ff1b": f(ff1_b), "ff2w": f(ff2_w),
        "ff2b": f(ff2_b), "wout": f(wout), "bout": f(bout),
        "pe2": _pe_table(), "masks": _masks(),
        "ident": np.eye(P, dtype=np.float32),
    }
    in_maps = []
    for c in range(NCORES):
        t = toks[c * BPC : (c + 1) * BPC].reshape(TOK)  # [512] flat tokens
        tokarr = np.ascontiguousarray(t.reshape(TC, P).T.astype(np.int32))
        in_maps.append({**shared, "tok": tokarr})
    return in_maps


def kernel(**inputs):
    if "nc" not in _CACHE:
        _CACHE["nc"] = _build()
    nc = _CACHE["nc"]
    in_maps = _prep_in_maps(**inputs)
    res = run_bass_kernel_spmd(nc, in_maps, core_ids=list(range(NCORES)))
    _CACHE["last_results"] = res
    out = np.concatenate([res.results[c]["probs"] for c in range(NCORES)], axis=0)
    return out.astype(np.float32)


# revision 13
# speedup vs baseline: 157.4275x; 4.5180x over previous
"""CheckersGPT dense transformer forward pass on 8 Trainium2 NeuronCores.

Strategy: pure data-parallel over the batch dim (16 batches -> 2 per core).
Each core runs the full 6-layer transformer on its 512 tokens (2 batches x
256 tokens) with all weights replicated. No collectives needed; the final
[2, 512] probability slices are concatenated on the host.

Numerics: matmul operands are bf16 (weights pre-converted on host; activation
operands rounded to bf16 on the PSUM->SBUF evacuation), accumulation is fp32
in PSUM, and all softmax / layernorm / residual math is fp32. Set
CKGPT_MM_DT=f32r or f32 for full-width fallbacks.

Layout convention per core (P=128 partitions):
  xT   [128, 4, 512]  : x transposed; chunk c holds embed dims [128c,128c+128),
                        free dim = 512 tokens. Used as matmul lhsT/rhs.
  xN   [128, 4, 512]  : x natural; chunk c holds tokens [128c,128c+128),
                        free dim = 512 embed. Used for residuals / LN (fp32).
All matmuls are out = lhsT.T @ rhs with contraction on the partition dim.
The last layer only computes Q/attention/FFN for the final token of each
batch (the head reads only x[:, -1, :]).
"""

import os
import numpy as np
from contextlib import ExitStack

import ml_dtypes
import concourse.bass as bass
import concourse.tile as tile
from concourse import bacc, mybir
from concourse.bass_utils import run_bass_kernel_spmd

F32 = mybir.dt.float32
BF16 = mybir.dt.bfloat16
I32 = mybir.dt.int32
AX = mybir.AxisListType
ALU = mybir.AluOpType
ACTF = mybir.ActivationFunctionType

V, E, L, H, B, T = 512, 512, 6, 8, 16, 256
NCORES = 8
BPC = B // NCORES          # batches per core
TOK = BPC * T              # tokens per core
P = 128
EC = E // P                # embed chunks of 128
TC = TOK // P              # token chunks of 128
NEG = -1e9
EPS = 1e-5

MODE = os.environ.get("CKGPT_MM_DT", "bf16")   # bf16 | f32r | f32
# dtype of matmul-operand SBUF tiles and of the weights in DRAM
MM_DT = {"bf16": BF16, "f32r": F32, "f32": F32}[MODE]
MM_CAST = mybir.dt.float32r if MODE == "f32r" else None
NP_WDT = ml_dtypes.bfloat16 if MODE == "bf16" else np.float32

_CACHE = {}


def _c(ap):
    """Cast an AP for matmul input (f32r mode only)."""
    return ap.bitcast(MM_CAST) if MM_CAST is not None else ap


def _mm(nc, out, lhsT, rhs, start, stop):
    nc.tensor.matmul(out, _c(lhsT), _c(rhs), start=start, stop=stop)


def _build(nlayers=L, reps=1, last_opt=True):
    nc = bacc.Bacc("TRN2", target_bir_lowering=False, debug=False, num_devices=NCORES)

    def din(name, shape, dtype=F32):
        return nc.dram_tensor(name, list(shape), dtype, kind="ExternalInput").ap()

    tok = din("tok", [P, TC], I32)            # token ids, p-major within chunks
    emb = din("emb", [V, E])
    pe2 = din("pe2", [TOK, E])                # positional encoding tiled over BPC
    wq = din("wq", [L, H, E, E], MM_DT)
    wk = din("wk", [L, H, E, E], MM_DT)
    wv = din("wv", [L, H, E, E], MM_DT)
    wo = din("wo", [L, H * E, E], MM_DT)
    bo = din("bo", [L, E])
    ln1w = din("ln1w", [L, E])
    ln1b = din("ln1b", [L, E])
    ln2w = din("ln2w", [L, E])
    ln2b = din("ln2b", [L, E])
    ff1w = din("ff1w", [L, E, E], MM_DT)
    ff1b = din("ff1b", [L, E])
    ff2w = din("ff2w", [L, E, E], MM_DT)
    ff2b = din("ff2b", [L, E])
    wout = din("wout", [E, V], MM_DT)
    bout = din("bout", [V])
    masks = din("masks", [2, P, T])           # additive causal mask per i-chunk
    ident = din("ident", [P, P])
    probs = nc.dram_tensor("probs", [BPC, V], F32, kind="ExternalOutput").ap()
    aps = (emb, pe2, wq, wk, wv, wo, bo, ln1w, ln1b, ln2w, ln2b,
           ff1w, ff1b, ff2w, ff2b, wout, bout, masks, ident, probs, tok)

    with tile.TileContext(nc) as tc, ExitStack() as ctx:
        if reps > 1:
            with tc.For_i(0, reps, 1):
                _emit(nc, tc, ctx, aps, nlayers, last_opt)
        else:
            _emit(nc, tc, ctx, aps, nlayers, last_opt)

    nc.compile()
    return nc


def _emit(nc, tc, ctx, aps, nlayers, last_opt):
    (emb, pe2, wq, wk, wv, wo, bo, ln1w, ln1b, ln2w, ln2b,
     ff1w, ff1b, ff2w, ff2b, wout, bout, masks, ident, probs, tok) = aps
    ep = ctx.enter_context

    const = ep(tc.tile_pool(name="const", bufs=1))
    wqkv_p = ep(tc.tile_pool(name="wqkv", bufs=4))
    wo_p = ep(tc.tile_pool(name="wo", bufs=2))
    wff_p = ep(tc.tile_pool(name="wff", bufs=1))
    bias_p = ep(tc.tile_pool(name="bias", bufs=1))
    act_p = ep(tc.tile_pool(name="act", bufs=2))
    qkv_p = ep(tc.tile_pool(name="qkvact", bufs=5))
    ot_p = ep(tc.tile_pool(name="ot", bufs=3))
    ff_p = ep(tc.tile_pool(name="ffact", bufs=3))
    tmp_p = ep(tc.tile_pool(name="tmp", bufs=3))
    esb_p = ep(tc.tile_pool(name="esb", bufs=6))
    attT_p = ep(tc.tile_pool(name="attT", bufs=3))
    st_p = ep(tc.tile_pool(name="stats", bufs=8))
    out_p = ep(tc.tile_pool(name="outp", bufs=1))

    ppb = ep(tc.tile_pool(name="ppb", bufs=3, space="PSUM"))
    ppa = ep(tc.tile_pool(name="ppa", bufs=3, space="PSUM"))
    ppt = ep(tc.tile_pool(name="ppt", bufs=2, space="PSUM"))

    # ---- constants ----
    ident_t = const.tile([P, P], F32)
    nc.sync.dma_start(out=ident_t[:], in_=ident)
    mask_t = const.tile([P, 2, T], F32)
    nc.sync.dma_start(out=mask_t[:], in_=masks.rearrange("c p j -> p c j"))
    eps_t = const.tile([P, 1], F32)
    nc.vector.memset(eps_t[:], EPS)
    tok_t = const.tile([P, TC], I32)
    nc.sync.dma_start(out=tok_t[:], in_=tok)

    def wtile(pool, dram2d, tag):
        t = pool.tile([P, EC, E], MM_DT, tag=tag)
        nc.sync.dma_start(
            out=_c(t[:]),
            in_=_c(dram2d.rearrange("(c p) o -> p c o", p=P)),
        )
        return t

    def bbcast(vec_ap, tag="bias"):
        t = bias_p.tile([P, E], F32, tag=tag)
        nc.sync.dma_start(out=t[:], in_=vec_ap.partition_broadcast(P))
        return t

    def evac(dst, src, use_act):
        """PSUM -> SBUF copy (dtype conversion happens on write)."""
        if use_act:
            nc.scalar.copy(_c(dst), src)
        else:
            nc.vector.tensor_copy(_c(dst), src)

    def transpose_into(dstT, srcN, nsrc_chunks=TC):
        # dstT[:, bb, a*P:(a+1)*P] = srcN[:, a, bb*P:(bb+1)*P].T
        for a in range(nsrc_chunks):
            for bb in range(EC):
                tp = ppt.tile([P, P], F32, tag="tp")
                nc.tensor.transpose(
                    tp[:], srcN[:, a, bb * P : (bb + 1) * P], ident_t[:]
                )
                evac(dstT[:, bb, a * P : (a + 1) * P], tp[:], (a + bb) % 2)

    def layernorm(src, dst, w_b, b_b, tag, rows=P):
        # dst = (src - mean)/sqrt(var+eps) * w + b ; src/dst [rows, E] fp32
        stt = st_p.tile([P, 6], F32, tag=tag + "s")
        nc.vector.bn_stats(out=stt[:rows], in_=src)
        mv = st_p.tile([P, 2], F32, tag=tag + "m")
        nc.vector.bn_aggr(out=mv[:rows], in_=stt[:rows])
        sd = st_p.tile([P, 1], F32, tag=tag + "d")
        nc.scalar.activation(
            out=sd[:rows], in_=mv[:rows, 1:2], func=ACTF.Sqrt, bias=eps_t[:rows, 0:1]
        )
        rs = st_p.tile([P, 1], F32, tag=tag + "r")
        nc.vector.reciprocal(out=rs[:rows], in_=sd[:rows])
        nc.vector.tensor_scalar(
            out=dst, in0=src, scalar1=mv[:rows, 0:1], scalar2=rs[:rows],
            op0=ALU.subtract, op1=ALU.mult,
        )
        nc.gpsimd.tensor_mul(out=dst, in0=dst, in1=w_b[:rows, :])
        nc.gpsimd.tensor_add(out=dst, in0=dst, in1=b_b[:rows, :])

    # ---- embedding gather + positional encoding ----
    xN = act_p.tile([P, TC, E], F32, tag="xN")
    for c in range(TC):
        nc.gpsimd.indirect_dma_start(
            out=xN[:, c, :], out_offset=None, in_=emb,
            in_offset=bass.IndirectOffsetOnAxis(ap=tok_t[:, c : c + 1], axis=0),
        )
    pe_t = ff_p.tile([P, TC, E], F32, tag="ff32")
    nc.sync.dma_start(out=pe_t[:], in_=pe2.rearrange("(c p) o -> p c o", p=P))
    for c in range(TC):
        nc.vector.tensor_add(out=xN[:, c, :], in0=xN[:, c, :], in1=pe_t[:, c, :])
    xT = act_p.tile([P, EC, TOK], MM_DT, tag="xT")
    transpose_into(xT, xN)

    for l in range(nlayers):
        last = last_opt and (l == L - 1) and (nlayers == L)
        bo_b = bbcast(bo[l], "b_bo")
        ln1w_b = bbcast(ln1w[l], "b_l1w")
        ln1b_b = bbcast(ln1b[l], "b_l1b")
        ln2w_b = bbcast(ln2w[l], "b_l2w")
        ln2b_b = bbcast(ln2b[l], "b_l2b")
        ff2b_b = bbcast(ff2b[l], "b_f2")
        ff1b_t = bias_p.tile([P, EC], F32, tag="b_f1")
        nc.sync.dma_start(out=ff1b_t[:], in_=ff1b[l].rearrange("(c p) -> p c", p=P))

        if not last:
            attn_acc = act_p.tile([P, TC, E], F32, tag="acc")
        else:
            pw_last = ppb.tile([BPC, E], F32, tag="ppb")

        for h in range(H):
            wq_t = wtile(wqkv_p, wq[l, h], "wqkv")
            wk_t = wtile(wqkv_p, wk[l, h], "wqkv")
            wv_t = wtile(wqkv_p, wv[l, h], "wqkv")
            wo_t = wtile(wo_p, wo[l, h * E : (h + 1) * E, :], "wo")

            # K^T: [o-chunk, tokens] ; V natural: [t-chunk, o]
            KT = qkv_p.tile([P, EC, TOK], MM_DT, tag="qkv")
            for oc in range(EC):
                ps = ppb.tile([P, TOK], F32, tag="ppb")
                for ec in range(EC):
                    _mm(nc, ps[:], wk_t[:, ec, oc * P : (oc + 1) * P],
                        xT[:, ec, :], ec == 0, ec == EC - 1)
                evac(KT[:, oc, :], ps[:], True)
            VN = qkv_p.tile([P, TC, E], MM_DT, tag="qkv")
            for tcc in range(TC):
                ps = ppb.tile([P, E], F32, tag="ppb")
                for ec in range(EC):
                    _mm(nc, ps[:], xT[:, ec, tcc * P : (tcc + 1) * P],
                        wv_t[:, ec, :], ec == 0, ec == EC - 1)
                evac(VN[:, tcc, :], ps[:], False)

            if not last:
                QT = qkv_p.tile([P, EC, TOK], MM_DT, tag="qkv")
                for oc in range(EC):
                    ps = ppb.tile([P, TOK], F32, tag="ppb")
                    for ec in range(EC):
                        _mm(nc, ps[:], wq_t[:, ec, oc * P : (oc + 1) * P],
                            xT[:, ec, :], ec == 0, ec == EC - 1)
                    evac(QT[:, oc, :], ps[:], True)

                for b in range(BPC):
                    t0 = b * T
                    # --- energy + softmax, causal-skipped ---
                    # i-chunk 0 attends j in [0,128); i-chunk 1 attends [0,256)
                    pse0 = ppa.tile([P, T], F32, tag="ppa")
                    for ec in range(EC):
                        _mm(nc, pse0[:, 0:P], QT[:, ec, (2 * b) * P : (2 * b + 1) * P],
                            KT[:, ec, t0 : t0 + P], ec == 0, ec == EC - 1)
                    pse1 = ppa.tile([P, T], F32, tag="ppa")
                    for ec in range(EC):
                        _mm(nc, pse1[:], QT[:, ec, (2 * b + 1) * P : (2 * b + 2) * P],
                            KT[:, ec, t0 : t0 + T], ec == 0, ec == EC - 1)

                    att0 = esb_p.tile([P, P], F32, tag="esb0")
                    nc.vector.tensor_add(
                        out=att0[:], in0=pse0[:, 0:P], in1=mask_t[:, 0, 0:P]
                    )
                    att1 = esb_p.tile([P, T], F32, tag="esb1")
                    nc.vector.tensor_add(out=att1[:], in0=pse1[:], in1=mask_t[:, 1, :])
                    for att in (att0, att1):
                        nmax = st_p.tile([P, 1], F32, tag="nmax")
                        nc.vector.reduce_max(
                            out=nmax[:], in_=att[:], axis=AX.X, negate=True
                        )
                        den = st_p.tile([P, 1], F32, tag="den")
                        nc.scalar.activation(
                            out=att[:], in_=att[:], func=ACTF.Exp,
                            bias=nmax[:, 0:1], accum_out=den[:],
                        )
                        rec = st_p.tile([P, 1], F32, tag="rec")
                        nc.vector.reciprocal(out=rec[:], in_=den[:])
                        nc.gpsimd.tensor_scalar_mul(
                            out=att[:], in0=att[:], scalar1=rec[:]
                        )

                    # --- transpose att blocks: (i0,j0), (i1,j0), (i1,j1) ---
                    attT = attT_p.tile([P, 2, T], MM_DT, tag="attT")
                    blocks = [(att0, 0, 0, 0), (att1, 0, 0, P), (att1, P, 1, P)]
                    for k, (src, joff, jc, ioff) in enumerate(blocks):
                        tp = ppt.tile([P, P], F32, tag="tp")
                        nc.tensor.transpose(
                            tp[:], src[:, joff : joff + P], ident_t[:]
                        )
                        evac(attT[:, jc, ioff : ioff + P], tp[:], k % 2)

                    # --- AV: o^T[e,i] accumulated over j-chunks ---
                    oTb = ot_p.tile([P, EC, T], MM_DT, tag="oT")
                    for ec in range(EC):
                        po = ppa.tile([P, T], F32, tag="ppa")
                        _mm(nc, po[:], VN[:, 2 * b, ec * P : (ec + 1) * P],
                            attT[:, 0, :], True, False)
                        _mm(nc, po[:, P:T], VN[:, 2 * b + 1, ec * P : (ec + 1) * P],
                            attT[:, 1, P:T], False, True)
                        evac(oTb[:, ec, :], po[:], False)

                    # --- wo partial for this (h, b) ---
                    for tcl in range(2):
                        tcc = 2 * b + tcl
                        pw = ppb.tile([P, E], F32, tag="ppb")
                        for ec in range(EC):
                            _mm(nc, pw[:], oTb[:, ec, tcl * P : (tcl + 1) * P],
                                wo_t[:, ec, :], ec == 0, ec == EC - 1)
                        if h == 0:
                            nc.vector.tensor_copy(attn_acc[:, tcc, :], pw[:])
                        else:
                            nc.vector.tensor_add(
                                out=attn_acc[:, tcc, :],
                                in0=attn_acc[:, tcc, :], in1=pw[:],
                            )
            else:
                # ---- last layer: only the final token of each batch ----
                QTl = qkv_p.tile([P, EC, BPC], MM_DT, tag="qtl")
                for oc in range(EC):
                    ps = ppt.tile([P, BPC], F32, tag="tp")
                    for ec in range(EC):
                        _mm(nc, ps[:], wq_t[:, ec, oc * P : (oc + 1) * P],
                            xT[:, ec, T - 1 :: T], ec == 0, ec == EC - 1)
                    evac(QTl[:, oc, :], ps[:], True)
                oTl = ot_p.tile([P, EC, BPC], MM_DT, tag="oTl")
                for b in range(BPC):
                    t0 = b * T
                    pse = ppa.tile([1, T], F32, tag="ppa")
                    for ec in range(EC):
                        _mm(nc, pse[:], QTl[:, ec, b : b + 1],
                            KT[:, ec, t0 : t0 + T], ec == 0, ec == EC - 1)
                    att = esb_p.tile([1, T], F32, tag="esbl")
                    nmax = st_p.tile([1, 1], F32, tag="nmaxl")
                    nc.vector.reduce_max(
                        out=nmax[:], in_=pse[:], axis=AX.X, negate=True
                    )
                    den = st_p.tile([1, 1], F32, tag="denl")
                    nc.scalar.activation(
                        out=att[:], in_=pse[:], func=ACTF.Exp,
                        bias=nmax[0:1, 0:1], accum_out=den[:],
                    )
                    rec = st_p.tile([1, 1], F32, tag="recl")
                    nc.vector.reciprocal(out=rec[:], in_=den[:])
                    nc.vector.tensor_scalar_mul(out=att[:], in0=att[:], scalar1=rec[:])
                    attTl = attT_p.tile([P, 2, 1], MM_DT, tag="attTl")
                    for jc in range(2):
                        tp = ppt.tile([P, 1], F32, tag="tp")
                        nc.tensor.transpose(
                            tp[:], att[0:1, jc * P : (jc + 1) * P],
                            ident_t[0:1, 0:1],
                        )
                        evac(attTl[:, jc, 0:1], tp[:], jc % 2)
                    for ec in range(EC):
                        po = ppt.tile([P, 1], F32, tag="tp")
                        _mm(nc, po[:], VN[:, 2 * b, ec * P : (ec + 1) * P],
                            attTl[:, 0, 0:1], True, False)
                        _mm(nc, po[:], VN[:, 2 * b + 1, ec * P : (ec + 1) * P],
                            attTl[:, 1, 0:1], False, True)
                        evac(oTl[:, ec, b : b + 1], po[:], False)
                # accumulate wo partials directly in PSUM across (h, ec)
                for ec in range(EC):
                    _mm(nc, pw_last[:], oTl[:, ec, :], wo_t[:, ec, :],
                        h == 0 and ec == 0, h == H - 1 and ec == EC - 1)

        # ---- FFN block ----
        ff1w_t = wtile(wff_p, ff1w[l], "wff1")
        ff2w_t = wtile(wff_p, ff2w[l], "wff2")

        if not last:
            h1N = ff_p.tile([P, TC, E], F32, tag="ff32")
            for tcc in range(TC):
                nc.gpsimd.tensor_add(
                    out=attn_acc[:, tcc, :], in0=attn_acc[:, tcc, :], in1=bo_b[:]
                )
                s1 = tmp_p.tile([P, E], F32, tag="s1")
                nc.gpsimd.tensor_add(
                    out=s1[:], in0=attn_acc[:, tcc, :], in1=xN[:, tcc, :]
                )
                layernorm(s1[:], h1N[:, tcc, :], ln1w_b, ln1b_b, "ln1")

            h1T = ff_p.tile([P, EC, TOK], MM_DT, tag="ffT")
            transpose_into(h1T, h1N)

            r1T = ff_p.tile([P, EC, TOK], MM_DT, tag="ffT")
            for fc in range(EC):
                ps = ppb.tile([P, TOK], F32, tag="ppb")
                for ec in range(EC):
                    _mm(nc, ps[:], ff1w_t[:, ec, fc * P : (fc + 1) * P],
                        h1T[:, ec, :], ec == 0, ec == EC - 1)
                nc.scalar.activation(
                    out=_c(r1T[:, fc, :]), in_=ps[:], func=ACTF.Relu,
                    bias=ff1b_t[:, fc : fc + 1],
                )

            xN_new = act_p.tile([P, TC, E], F32, tag="xN")
            for tcc in range(TC):
                ps = ppb.tile([P, E], F32, tag="ppb")
                for fc in range(EC):
                    _mm(nc, ps[:], r1T[:, fc, tcc * P : (tcc + 1) * P],
                        ff2w_t[:, fc, :], fc == 0, fc == EC - 1)
                s2 = tmp_p.tile([P, E], F32, tag="s1")
                nc.vector.tensor_add(out=s2[:], in0=ps[:], in1=ff2b_b[:])
                nc.gpsimd.tensor_add(out=s2[:], in0=s2[:], in1=attn_acc[:, tcc, :])
                layernorm(s2[:], xN_new[:, tcc, :], ln2w_b, ln2b_b, "ln2")
            xN = xN_new
            xT = act_p.tile([P, EC, TOK], MM_DT, tag="xT")
            transpose_into(xT, xN)
        else:
            # ---- last layer FFN on 2 tokens only ----
            ao_l = out_p.tile([BPC, E], F32)
            nc.vector.tensor_add(out=ao_l[:], in0=pw_last[:], in1=bo_b[:BPC, :])
            x_l = out_p.tile([BPC, E], F32)
            for b in range(BPC):
                nc.sync.dma_start(
                    out=x_l[b : b + 1, :], in_=xN[P - 1 : P, 2 * b + 1, :]
                )
            s1 = out_p.tile([BPC, E], F32)
            nc.vector.tensor_add(out=s1[:], in0=ao_l[:], in1=x_l[:])
            h1_l = out_p.tile([BPC, E], F32)
            layernorm(s1[:], h1_l[:], ln1w_b, ln1b_b, "lnL1", rows=BPC)
            h1T_l = ff_p.tile([P, EC, BPC], MM_DT, tag="h1Tl")
            for bb in range(EC):
                tp = ppt.tile([P, BPC], F32, tag="tp")
                nc.tensor.transpose(
                    tp[:], h1_l[:, bb * P : (bb + 1) * P],
                    ident_t[0:BPC, 0:BPC],
                )
                evac(h1T_l[:, bb, :], tp[:], bb % 2)
            r1T_l = ff_p.tile([P, EC, BPC], MM_DT, tag="r1Tl")
            for fc in range(EC):
                ps = ppt.tile([P, BPC], F32, tag="tp")
                for ec in range(EC):
                    _mm(nc, ps[:], ff1w_t[:, ec, fc * P : (fc + 1) * P],
                        h1T_l[:, ec, :], ec == 0, ec == EC - 1)
                nc.scalar.activation(
                    out=_c(r1T_l[:, fc, :]), in_=ps[:], func=ACTF.Relu,
                    bias=ff1b_t[:, fc : fc + 1],
                )
            ps2 = ppb.tile([BPC, E], F32, tag="ppb")
            for fc in range(EC):
                _mm(nc, ps2[:], r1T_l[:, fc, :], ff2w_t[:, fc, :],
                    fc == 0, fc == EC - 1)
            s2 = out_p.tile([BPC, E], F32)
            nc.vector.tensor_add(out=s2[:], in0=ps2[:], in1=ff2b_b[:BPC, :])
            nc.vector.tensor_add(out=s2[:], in0=s2[:], in1=ao_l[:])
            xl = out_p.tile([BPC, E], F32)
            layernorm(s2[:], xl[:], ln2w_b, ln2b_b, "lnL2", rows=BPC)
            xlT = ff_p.tile([P, EC, BPC], MM_DT, tag="xlT")
            for bb in range(EC):
                tp = ppt.tile([P, BPC], F32, tag="tp")
                nc.tensor.transpose(
                    tp[:], xl[:, bb * P : (bb + 1) * P],
                    ident_t[0:BPC, 0:BPC],
                )
                evac(xlT[:, bb, :], tp[:], bb % 2)

    # ---- output head: last token of each batch ----
    wout_t = wtile(wo_p, wout, "wo")
    bout_t = out_p.tile([BPC, V], F32)
    nc.sync.dma_start(out=bout_t[:], in_=bout.partition_broadcast(BPC))
    pl = ppb.tile([BPC, V], F32, tag="ppb")
    if nlayers == L and last_opt:
        xl_lhs = xlT
        cols = slice(0, BPC)
    else:
        xl_lhs = xT
        cols = slice(T - 1, TOK, T)
    for ec in range(EC):
        _mm(nc, pl[:], xl_lhs[:, ec, cols], wout_t[:, ec, :], ec == 0, ec == EC - 1)
    logits = out_p.tile([BPC, V], F32)
    nc.vector.tensor_add(out=logits[:], in0=pl[:], in1=bout_t[:])
    nmax = out_p.tile([BPC, 1], F32)
    nc.vector.reduce_max(out=nmax[:], in_=logits[:], axis=AX.X, negate=True)
    den = out_p.tile([BPC, 1], F32)
    nc.scalar.activation(
        out=logits[:], in_=logits[:], func=ACTF.Exp,
        bias=nmax[:, 0:1], accum_out=den[:],
    )
    rec = out_p.tile([BPC, 1], F32)
    nc.vector.reciprocal(out=rec[:], in_=den[:])
    nc.vector.tensor_scalar_mul(out=logits[:], in0=logits[:], scalar1=rec[:])
    nc.sync.dma_start(out=probs, in_=logits[:])


def _pe_table():
    i = np.arange(E, dtype=np.float32)
    rates = (1.0 / np.power(np.float32(10000.0), 2.0 * np.floor(i / 2.0) / E)).astype(
        np.float32
    )
    ang = np.arange(T, dtype=np.float32)[:, None] * rates[None, :]
    pe = np.concatenate([np.sin(ang[:, 0::2]), np.cos(ang[:, 1::2])], axis=-1)
    return np.tile(pe.astype(np.float32), (BPC, 1))  # [TOK, E]


def _masks():
    m = np.zeros((2, P, T), dtype=np.float32)
    j = np.arange(T)
    for c in range(2):
        i = c * P + np.arange(P)
        m[c] = np.where(j[None, :] > i[:, None], np.float32(NEG), np.float32(0.0))
    return m


def _prep_in_maps(
    input_tokens, emb, wq, wk, wv, wo, bo, ln1_w, ln1_b, ln2_w, ln2_b,
    ff1_w, ff1_b, ff2_w, ff2_b, wout, bout,
):
    f = lambda x: np.ascontiguousarray(np.asarray(x, dtype=np.float32))
    w = lambda x: np.ascontiguousarray(np.asarray(x, dtype=np.float32).astype(NP_WDT))
    toks = np.asarray(input_tokens).astype(np.int64)
    shared = {
        "emb": f(emb), "wq": w(wq), "wk": w(wk), "wv": w(wv), "wo": w(wo),
        "bo": f(bo), "ln1w": f(ln1_w), "ln1b": f(ln1_b), "ln2w": f(ln2_w),
        "ln2b": f(ln2_b), "ff1w": w(ff1_w), "ff1b": f(ff1_b), "ff2w": w(ff2_w),
        "ff2b": f(ff2_b), "wout": w(wout), "bout": f(bout),
        "pe2": _pe_table(), "masks": _masks(),
        "ident": np.eye(P, dtype=np.float32),
    }
    in_maps = []
    for c in range(NCORES):
        t = toks[c * BPC : (c + 1) * BPC].reshape(TOK)  # [512] flat tokens
        tokarr = np.ascontiguousarray(t.reshape(TC, P).T.astype(np.int32))
        in_maps.append({**shared, "tok": tokarr})
    return in_maps


def kernel(**inputs):
    if "nc" not in _CACHE:
        _CACHE["nc"] = _build()
    nc = _CACHE["nc"]
    in_maps = _prep_in_maps(**inputs)
    res = run_bass_kernel_spmd(nc, in_maps, core_ids=list(range(NCORES)))
    _CACHE["last_results"] = res
    out = np.concatenate([res.results[c]["probs"] for c in range(NCORES)], axis=0)
    return out.astype(np.float32)
